# revision 20
# baseline (speedup 1.0000x reference)
"""Trainium2 Bass kernel for the DialogueGNN gated multimodal fusion layer.

Computes, for N = B*L nodes (node n = b*L + t, batch-major flatten):
    ha = tanh(na @ Wa.T + ba)   (same for hv, hl)
    z_xy = sigmoid([nx, ny, nx*ny] @ Wxy.T + bxy)    for xy in {av, al, vl}
    h_xy = z_xy * hx + (1 - z_xy) * hy
    out  = concat([h_av, h_al, h_vl], axis=-1)       # (N, 3D) fp32

Strategy (8 NeuronCores, data-parallel over nodes):
  * Host: shard batches 16-per-core, pre-transpose activations to
    feature-major [2, 128, 16384] fp16; upcast the fp16 device output to
    fp32 on the host (exact).
  * Device, per 1024-node chunk (activations DMA'd 2 chunks at a time,
    4 KB descriptors):
      - DVE products na*nv etc. (feature-major fp16, gate bilinear terms),
      - PE: activations stationary, [128,256] weight tile streams as rhs,
        plus 3-col gate rhs reusing the stationary; products stream their
        own 3-col gate rhs; all gate partials accumulate in one PSUM tile,
      - ACT: tanh/sigmoid drains of PSUM (fp16 out),
      - fusion h = z*hx + (1-z)*hy as t = z*hx (tensor_scalar 4x mode,
        partially on ACT for engine balance) then one fused
        scalar_tensor_tensor h = (hy*(1-z)) + t per 128-node tile,
      - DMA out [128, 8, 768] fp16 -> node-major rows (1536 B descriptors).
"""

import os
import sys
from contextlib import ExitStack

import numpy as np

for _p in ("/opt/trn_rl_repo", "/root/.axon_site/_ro/trn_rl_repo"):
    if os.path.isdir(_p) and _p not in sys.path:
        sys.path.insert(0, _p)

import concourse.bass as bass
import concourse.bacc as bacc
import concourse.tile as tile
from concourse import mybir
from concourse.bass_utils import run_bass_kernel_spmd

L, B, D = 1024, 128, 256
N_CORES = 8
B_CORE = B // N_CORES          # 16 batches per core
N_CORE = B_CORE * L            # 16384 nodes per core
CHUNK = 1024                   # nodes per compute chunk
SUPER = 2 * CHUNK              # nodes per input-DMA superchunk
NTILE = CHUNK // 128           # 8 node-tiles of 128 per chunk
NSUPER = N_CORE // SUPER       # 8 superchunks per core

MM_DT = mybir.dt.float16       # matmul / elementwise-intermediate dtype
NP_MM_DT = np.float16

F32 = mybir.dt.float32
AX = mybir.AluOpType
AF = mybir.ActivationFunctionType

# pairs: (o, x, y) with h_o = z_o * h_x + (1 - z_o) * h_y
PAIRS = ((0, 0, 1), (1, 0, 2), (2, 1, 2))
# Routing pattern for the 24 per-chunk t = z*d scaling instructions:
# cycled per instruction; "a" = ACT engine, "p" = GpSimd/Pool, "v" = DVE.
# DVE carries the chunk-wide tensor ops, so scalings go to ACT/Pool.
TS_ROUTE = os.environ.get("TS_ROUTE", "avavvavavvav")


def _build_nc(with_bias: bool):
    """Build the Bass program (identical on all 8 cores)."""
    nc = bacc.Bacc("TRN2", target_bir_lowering=False, debug=False)

    xa = nc.dram_tensor("a_t", [2, 128, N_CORE], MM_DT, kind="ExternalInput")
    xv = nc.dram_tensor("v_t", [2, 128, N_CORE], MM_DT, kind="ExternalInput")
    xl = nc.dram_tensor("l_t", [2, 128, N_CORE], MM_DT, kind="ExternalInput")
    wm = nc.dram_tensor("w_main", [3, 2, 128, D], MM_DT, kind="ExternalInput")
    wg = nc.dram_tensor("w_gate", [6, 2, 128, 3], MM_DT, kind="ExternalInput")
    if with_bias:
        mb = nc.dram_tensor("b_main", [1, 3, D], MM_DT, kind="ExternalInput")
        gb = nc.dram_tensor("b_gate", [1, 3], MM_DT, kind="ExternalInput")
    out = nc.dram_tensor("out", [N_CORE, 3 * D], MM_DT, kind="ExternalOutput")

    with tile.TileContext(nc) as tc, ExitStack() as ctx:
        const = ctx.enter_context(tc.tile_pool(name="const", bufs=1))
        io_in = ctx.enter_context(tc.tile_pool(name="io_in", bufs=2))
        prod_p = ctx.enter_context(tc.tile_pool(name="prod", bufs=2))
        h_p = ctx.enter_context(tc.tile_pool(name="h", bufs=2))
        t_p = ctx.enter_context(tc.tile_pool(name="t", bufs=3))
        z_p = ctx.enter_context(tc.tile_pool(name="z", bufs=3))
        out_p = ctx.enter_context(tc.tile_pool(name="out", bufs=3))
        ps_main = ctx.enter_context(
            tc.tile_pool(name="ps_main", bufs=3, space="PSUM"))
        ps_z = ctx.enter_context(tc.tile_pool(name="ps_z", bufs=2, space="PSUM"))

        # ---- constants ----
        w_main_sb = const.tile([128, 6, D], MM_DT)       # (mod, kh) -> idx m*2+kh
        nc.sync.dma_start(out=w_main_sb,
                          in_=wm.rearrange("m k p c -> p (m k) c"))
        w_gate_sb = const.tile([128, 12, 3], MM_DT)      # (stream, kh) -> s*2+kh
        nc.sync.dma_start(out=w_gate_sb,
                          in_=wg.rearrange("s k p c -> p (s k) c"))
        if with_bias:
            ones_sb = const.tile([1, 128], MM_DT)
            nc.vector.memset(ones_sb, 1.0)
            mb_sb = const.tile([1, 3, D], MM_DT)
            nc.sync.dma_start(out=mb_sb, in_=mb)
            gb_sb = const.tile([1, 3], MM_DT)
            nc.sync.dma_start(out=gb_sb, in_=gb)

        for sc in range(NSUPER):
            ssl = slice(sc * SUPER, (sc + 1) * SUPER)

            # ---- load feature-major activations (2 chunks, 4 KB rows) ----
            na2 = io_in.tile([128, 2, SUPER], MM_DT, tag="na")
            nv2 = io_in.tile([128, 2, SUPER], MM_DT, tag="nv")
            nl2 = io_in.tile([128, 2, SUPER], MM_DT, tag="nl")
            for t_sb, t_dr in ((na2, xa), (nv2, xv), (nl2, xl)):
                nc.sync.dma_start(
                    out=t_sb,
                    in_=t_dr[:, :, ssl].rearrange("k p n -> p k n"))

            # ---- pairwise products (gate bilinear terms), both chunks ----
            prods2 = []
            for g, (x, y) in enumerate(((na2, nv2), (na2, nl2), (nv2, nl2))):
                pg = prod_p.tile([128, 2, SUPER], MM_DT, tag=f"p{g}")
                nc.vector.tensor_mul(pg, x, y)
                prods2.append(pg)

            for cc in range(2):
                ch = sc * 2 + cc
                csl = slice(ch * CHUNK, (ch + 1) * CHUNK)
                # per-chunk views, node-tile j = nodes [j*128, (j+1)*128)
                acts = [t2.rearrange("p k (c j n) -> p k c j n", c=2, j=NTILE)
                        for t2 in (na2, nv2, nl2)]
                prods = [pg.rearrange("p k (c j n) -> p k c j n", c=2, j=NTILE)
                         for pg in prods2]

                # ---- matmuls: activations stationary, weights moving ----
                z_ps = ps_z.tile([128, 3 * NTILE], F32)
                # product-gate matmuls, interleaved into the m>=1 main
                # groups below so their stationary loads hide behind the
                # 256-col main streams and z_ps completes early.
                pg_specs = [(g, j, kh)
                            for g in range(3)
                            for j in range(NTILE)
                            for kh in range(2)]

                def emit_pg(stop):
                    g, j, kh = pg_specs.pop(0)
                    nc.tensor.matmul(
                        z_ps[:, j * 3:(j + 1) * 3],
                        lhsT=prods[g][:, kh, cc, j, :],
                        rhs=w_gate_sb[:, (3 + g) * 2 + kh, :],
                        start=False, stop=stop, skip_group_check=True,
                    )

                hs = []
                for m in range(3):
                    src = acts[m]
                    h_m = h_p.tile([128, NTILE * D], MM_DT, tag=f"h{m}")
                    for half in range(2):
                        ps = ps_main.tile([128, 4 * D], F32, tag="hps")
                        for jj in range(4):
                            j = half * 4 + jj
                            for kh in range(2):
                                lhs = src[:, kh, cc, j, :]
                                nc.tensor.matmul(
                                    ps[:, jj * D:(jj + 1) * D],
                                    lhsT=lhs, rhs=w_main_sb[:, m * 2 + kh, :],
                                    start=(kh == 0),
                                    stop=(kh == 1 and not with_bias),
                                )
                                nc.tensor.matmul(
                                    z_ps[:, j * 3:(j + 1) * 3],
                                    lhsT=lhs, rhs=w_gate_sb[:, m * 2 + kh, :],
                                    start=(m == 0 and kh == 0 and j == 0),
                                    stop=False,
                                    skip_group_check=True,
                                )
                                if m >= 1 and len(pg_specs) > 16:
                                    emit_pg(stop=False)
                            if with_bias:
                                nc.tensor.matmul(
                                    ps[:, jj * D:(jj + 1) * D],
                                    lhsT=ones_sb, rhs=mb_sb[:, m, :],
                                    start=False, stop=True,
                                )
                        # tanh drain PSUM -> SBUF (fp16 out)
                        nc.scalar.activation(
                            out=h_m[:, half * 4 * D:(half + 1) * 4 * D], in_=ps,
                            func=AF.Tanh)
                    hs.append(h_m)

                # remaining gate contributions from the products
                while pg_specs:
                    emit_pg(stop=(len(pg_specs) == 1 and not with_bias))
                if with_bias:
                    for j in range(NTILE):
                        nc.tensor.matmul(
                            z_ps[:, j * 3:(j + 1) * 3], lhsT=ones_sb, rhs=gb_sb,
                            start=False, stop=True, skip_group_check=True,
                        )

                z_sb = z_p.tile([128, 3 * NTILE], F32, tag="z")
                nc.scalar.activation(out=z_sb, in_=z_ps, func=AF.Sigmoid)

                # ---- gated fusion: h = z*(hx - hy) + hy ----
                #   d = hx - hy        (chunk-wide tensor_tensor, 2x mode)
                #   t = z*d            (per-tile tensor_scalar, split DVE/ACT)
                #   h = t + hy         (chunk-wide tensor_tensor, 2x mode)
                h16 = out_p.tile([128, NTILE, 3 * D], MM_DT, tag="h16")
                n_ts = 0
                for o, (zi, xi, yi) in enumerate(PAIRS):
                    hx, hy = hs[xi], hs[yi]
                    dg = t_p.tile([128, NTILE * D], MM_DT, tag="d")
                    nc.vector.tensor_sub(dg, hx, hy)
                    tg = t_p.tile([128, NTILE * D], MM_DT, tag="t")
                    for j in range(NTILE):
                        zcol = z_sb[:, j * 3 + o: j * 3 + o + 1]
                        route = TS_ROUTE[n_ts % len(TS_ROUTE)]
                        if route == "a":
                            nc.scalar.activation(
                                out=tg[:, j * D:(j + 1) * D],
                                in_=dg[:, j * D:(j + 1) * D],
                                func=AF.Copy, scale=zcol)
                        elif route == "p":
                            nc.gpsimd.tensor_scalar_mul(
                                tg[:, j * D:(j + 1) * D],
                                dg[:, j * D:(j + 1) * D],
                                zcol)
                        else:
                            nc.vector.tensor_scalar_mul(
                                tg[:, j * D:(j + 1) * D],
                                dg[:, j * D:(j + 1) * D],
                                zcol)
                        n_ts += 1
                    nc.vector.tensor_add(
                        h16[:, :, o * D:(o + 1) * D],
                        tg.rearrange("p (j d) -> p j d", d=D),
                        hy.rearrange("p (j d) -> p j d", d=D))

                nc.sync.dma_start(
                    out=out[csl, :].rearrange("(j p) c -> p j c", p=128),
                    in_=h16)

    nc.compile()
    return nc


_CACHE = {}


def _get_nc(with_bias: bool):
    key = ("nc", with_bias)
    if key not in _CACHE:
        _CACHE[key] = _build_nc(with_bias)
    return _CACHE[key]


def _prep_weights(Wa, Wv, Wl, Wav, Wal, Wvl):
    # w_main[m, kh] = W.T[kh*128:(kh+1)*128, :]  ([128, D] slice of [K, M])
    wm = np.stack([
        np.ascontiguousarray(W.T.reshape(2, 128, D))
        for W in (Wa, Wv, Wl)
    ]).astype(NP_MM_DT)                               # [3, 2, 128, D]
    # gate vectors, split into per-stream blocks of 3 columns
    wav, wal, wvl = Wav[0], Wal[0], Wvl[0]            # (768,)
    Z = np.zeros(D, np.float32)
    blocks = [
        (wav[0:D],      wal[0:D],      Z),            # stream na
        (wav[D:2 * D],  Z,             wvl[0:D]),     # stream nv
        (Z,             wal[D:2 * D],  wvl[D:2 * D]),  # stream nl
        (wav[2 * D:],   Z,             Z),            # stream na*nv
        (Z,             wal[2 * D:],   Z),            # stream na*nl
        (Z,             Z,             wvl[2 * D:]),  # stream nv*nl
    ]
    wg = np.stack([
        np.stack([np.asarray(c0), np.asarray(c1), np.asarray(c2)], axis=1)
        .reshape(2, 128, 3)
        for (c0, c1, c2) in blocks
    ]).astype(NP_MM_DT)                               # [6, 2, 128, 3]
    return wm, wg


def _prep_acts(x, c):
    """x: (L, B, D) fp32 -> core-c feature-major [2, 128, N_CORE] fp16."""
    xc = x[:, c * B_CORE:(c + 1) * B_CORE, :]         # (L, 16, D)
    xt = np.ascontiguousarray(xc.astype(NP_MM_DT).transpose(2, 1, 0))
    return xt.reshape(2, 128, N_CORE)                 # k-major, n = b*L + t


def kernel(**inputs) -> np.ndarray:
    a = np.asarray(inputs["a"], np.float32)
    v = np.asarray(inputs["v"], np.float32)
    l = np.asarray(inputs["l"], np.float32)
    names = ("Wa", "Wv", "Wl", "Wav", "Wal", "Wvl")
    Wa, Wv, Wl, Wav, Wal, Wvl = (np.asarray(inputs[n], np.float32)
                                 for n in names)
    biases = {n: np.asarray(inputs[n], np.float32)
              for n in ("ba", "bv", "bl", "bav", "bal", "bvl")}
    with_bias = any(np.any(b) for b in biases.values())

    nc = _get_nc(with_bias)
    wm, wg = _prep_weights(Wa, Wv, Wl, Wav, Wal, Wvl)

    in_maps = []
    for c in range(N_CORES):
        m = {
            "a_t": _prep_acts(a, c),
            "v_t": _prep_acts(v, c),
            "l_t": _prep_acts(l, c),
            "w_main": wm,
            "w_gate": wg,
        }
        if with_bias:
            m["b_main"] = np.stack(
                [biases["ba"], biases["bv"], biases["bl"]])[None].astype(NP_MM_DT)
            m["b_gate"] = np.array(
                [[biases["bav"][0], biases["bal"][0], biases["bvl"][0]]],
                NP_MM_DT)
        in_maps.append(m)

    trace = bool(int(os.environ.get("KERNEL_TRACE", "0")))
    kw = {}
    if trace and os.environ.get("KERNEL_TRACE_DIR"):
        kw["tmpdir"] = os.environ["KERNEL_TRACE_DIR"]
    res = run_bass_kernel_spmd(nc, in_maps, core_ids=list(range(N_CORES)),
                               trace=trace, **kw)
    _CACHE["last_results"] = res
    return np.concatenate([res.results[c]["out"] for c in range(N_CORES)],
                          axis=0).astype(np.float32)


# revision 23
# speedup vs baseline: 1.2127x; 1.2127x over previous
"""Trainium2 Bass kernel for the DialogueGNN gated multimodal fusion layer.

Computes, for N = B*L nodes (node n = b*L + t, batch-major flatten):
    ha = tanh(na @ Wa.T + ba)   (same for hv, hl)
    z_xy = sigmoid([nx, ny, nx*ny] @ Wxy.T + bxy)    for xy in {av, al, vl}
    h_xy = z_xy * hx + (1 - z_xy) * hy
    out  = concat([h_av, h_al, h_vl], axis=-1)       # (N, 3D) fp32

Strategy (8 NeuronCores, data-parallel over nodes):
  * Host: shard batches 16-per-core, pre-transpose activations to
    feature-major [2, 128, 16384] fp16; upcast the fp16 device output to
    fp32 on the host (exact).
  * Device, per 1024-node chunk (activations DMA'd 2 chunks at a time,
    4 KB descriptors):
      - DVE products na*nv etc. (feature-major fp16, gate bilinear terms),
      - PE: activations stationary, [128,256] weight tile streams as rhs,
        plus 3-col gate rhs reusing the stationary; products stream their
        own 3-col gate rhs; all gate partials accumulate in one PSUM tile,
      - ACT: tanh/sigmoid drains of PSUM (fp16 out),
      - fusion h = z*hx + (1-z)*hy as t = z*hx (tensor_scalar 4x mode,
        partially on ACT for engine balance) then one fused
        scalar_tensor_tensor h = (hy*(1-z)) + t per 128-node tile,
      - DMA out [128, 8, 768] fp16 -> node-major rows (1536 B descriptors).
"""

import os
import sys
from contextlib import ExitStack

import numpy as np

for _p in ("/opt/trn_rl_repo", "/root/.axon_site/_ro/trn_rl_repo"):
    if os.path.isdir(_p) and _p not in sys.path:
        sys.path.insert(0, _p)

import concourse.bass as bass
import concourse.bacc as bacc
import concourse.tile as tile
from concourse import mybir
from concourse.bass_utils import run_bass_kernel_spmd

L, B, D = 1024, 128, 256
N_CORES = 8
B_CORE = B // N_CORES          # 16 batches per core
N_CORE = B_CORE * L            # 16384 nodes per core
CHUNK = 1024                   # nodes per compute chunk
SUPER = 2 * CHUNK              # nodes per input-DMA superchunk
NTILE = CHUNK // 128           # 8 node-tiles of 128 per chunk
NSUPER = N_CORE // SUPER       # 8 superchunks per core

MM_DT = mybir.dt.float16       # matmul / elementwise-intermediate dtype
NP_MM_DT = np.float16

F32 = mybir.dt.float32
AX = mybir.AluOpType
AF = mybir.ActivationFunctionType

# pairs: (o, x, y) with h_o = z_o * h_x + (1 - z_o) * h_y
PAIRS = ((0, 0, 1), (1, 0, 2), (2, 1, 2))
# Routing pattern for the 24 per-chunk t = z*d scaling instructions:
# cycled per instruction; "a" = ACT engine, "p" = GpSimd/Pool, "v" = DVE.
# DVE carries the chunk-wide tensor ops, so scalings go to ACT/Pool.
TS_ROUTE = os.environ.get("TS_ROUTE", "avavvavavvav")


def _build_nc(with_bias: bool):
    """Build the Bass program (identical on all 8 cores)."""
    nc = bacc.Bacc("TRN2", target_bir_lowering=False, debug=False)

    xa = nc.dram_tensor("a_t", [2, 128, N_CORE], MM_DT, kind="ExternalInput")
    xv = nc.dram_tensor("v_t", [2, 128, N_CORE], MM_DT, kind="ExternalInput")
    xl = nc.dram_tensor("l_t", [2, 128, N_CORE], MM_DT, kind="ExternalInput")
    wm = nc.dram_tensor("w_main", [3, 2, 128, D], MM_DT, kind="ExternalInput")
    wg = nc.dram_tensor("w_gate", [6, 2, 128, 3], MM_DT, kind="ExternalInput")
    if with_bias:
        mb = nc.dram_tensor("b_main", [1, 3, D], MM_DT, kind="ExternalInput")
        gb = nc.dram_tensor("b_gate", [1, 3], MM_DT, kind="ExternalInput")
    out = nc.dram_tensor("out", [N_CORE, 3 * D], MM_DT, kind="ExternalOutput")

    with tile.TileContext(nc) as tc, ExitStack() as ctx:
        const = ctx.enter_context(tc.tile_pool(name="const", bufs=1))
        io_in = ctx.enter_context(tc.tile_pool(name="io_in", bufs=2))
        prod_p = ctx.enter_context(tc.tile_pool(name="prod", bufs=2))
        h_p = ctx.enter_context(tc.tile_pool(name="h", bufs=2))
        t_p = ctx.enter_context(tc.tile_pool(name="t", bufs=3))
        z_p = ctx.enter_context(tc.tile_pool(name="z", bufs=3))
        out_p = ctx.enter_context(tc.tile_pool(name="out", bufs=3))
        ps_main = ctx.enter_context(
            tc.tile_pool(name="ps_main", bufs=3, space="PSUM"))
        ps_z = ctx.enter_context(tc.tile_pool(name="ps_z", bufs=2, space="PSUM"))

        # ---- constants ----
        w_main_sb = const.tile([128, 6, D], MM_DT)       # (mod, kh) -> idx m*2+kh
        nc.sync.dma_start(out=w_main_sb,
                          in_=wm.rearrange("m k p c -> p (m k) c"))
        w_gate_sb = const.tile([128, 12, 3], MM_DT)      # (stream, kh) -> s*2+kh
        nc.sync.dma_start(out=w_gate_sb,
                          in_=wg.rearrange("s k p c -> p (s k) c"))
        if with_bias:
            ones_sb = const.tile([1, 128], MM_DT)
            nc.vector.memset(ones_sb, 1.0)
            mb_sb = const.tile([1, 3, D], MM_DT)
            nc.sync.dma_start(out=mb_sb, in_=mb)
            gb_sb = const.tile([1, 3], MM_DT)
            nc.sync.dma_start(out=gb_sb, in_=gb)

        for sc in range(NSUPER):
            ssl = slice(sc * SUPER, (sc + 1) * SUPER)

            # ---- load feature-major activations (2 chunks, 4 KB rows) ----
            na2 = io_in.tile([128, 2, SUPER], MM_DT, tag="na")
            nv2 = io_in.tile([128, 2, SUPER], MM_DT, tag="nv")
            nl2 = io_in.tile([128, 2, SUPER], MM_DT, tag="nl")
            for t_sb, t_dr in ((na2, xa), (nv2, xv), (nl2, xl)):
                nc.sync.dma_start(
                    out=t_sb,
                    in_=t_dr[:, :, ssl].rearrange("k p n -> p k n"))

            # ---- pairwise products (gate bilinear terms), both chunks ----
            prods2 = []
            for g, (x, y) in enumerate(((na2, nv2), (na2, nl2), (nv2, nl2))):
                pg = prod_p.tile([128, 2, SUPER], MM_DT, tag=f"p{g}")
                nc.vector.tensor_mul(pg, x, y)
                prods2.append(pg)

            for cc in range(2):
                ch = sc * 2 + cc
                csl = slice(ch * CHUNK, (ch + 1) * CHUNK)
                # per-chunk views, node-tile j = nodes [j*128, (j+1)*128)
                acts = [t2.rearrange("p k (c j n) -> p k c j n", c=2, j=NTILE)
                        for t2 in (na2, nv2, nl2)]
                prods = [pg.rearrange("p k (c j n) -> p k c j n", c=2, j=NTILE)
                         for pg in prods2]

                # ---- matmuls: activations stationary, weights moving ----
                z_ps = ps_z.tile([128, 3 * NTILE], F32)
                hs = []
                for m in range(3):
                    src = acts[m]
                    h_m = h_p.tile([128, NTILE * D], MM_DT, tag=f"h{m}")
                    for half in range(2):
                        ps = ps_main.tile([128, 4 * D], F32, tag="hps")
                        for jj in range(4):
                            j = half * 4 + jj
                            for kh in range(2):
                                lhs = src[:, kh, cc, j, :]
                                nc.tensor.matmul(
                                    ps[:, jj * D:(jj + 1) * D],
                                    lhsT=lhs, rhs=w_main_sb[:, m * 2 + kh, :],
                                    start=(kh == 0),
                                    stop=(kh == 1 and not with_bias),
                                )
                                nc.tensor.matmul(
                                    z_ps[:, j * 3:(j + 1) * 3],
                                    lhsT=lhs, rhs=w_gate_sb[:, m * 2 + kh, :],
                                    start=(m == 0 and kh == 0 and j == 0),
                                    stop=False,
                                    skip_group_check=True,
                                )
                            if with_bias:
                                nc.tensor.matmul(
                                    ps[:, jj * D:(jj + 1) * D],
                                    lhsT=ones_sb, rhs=mb_sb[:, m, :],
                                    start=False, stop=True,
                                )
                        # tanh drain PSUM -> SBUF (fp16 out)
                        nc.scalar.activation(
                            out=h_m[:, half * 4 * D:(half + 1) * 4 * D], in_=ps,
                            func=AF.Tanh)
                    hs.append(h_m)

                # gate contributions from the products
                for g in range(3):
                    pgv = prods[g]
                    for j in range(NTILE):
                        for kh in range(2):
                            last = (g == 2 and kh == 1 and not with_bias)
                            nc.tensor.matmul(
                                z_ps[:, j * 3:(j + 1) * 3],
                                lhsT=pgv[:, kh, cc, j, :],
                                rhs=w_gate_sb[:, (3 + g) * 2 + kh, :],
                                start=False, stop=last, skip_group_check=True,
                            )
                if with_bias:
                    for j in range(NTILE):
                        nc.tensor.matmul(
                            z_ps[:, j * 3:(j + 1) * 3], lhsT=ones_sb, rhs=gb_sb,
                            start=False, stop=True, skip_group_check=True,
                        )

                z_sb = z_p.tile([128, 3 * NTILE], F32, tag="z")
                nc.scalar.activation(out=z_sb, in_=z_ps, func=AF.Sigmoid)

                # ---- gated fusion: h = z*(hx - hy) + hy ----
                #   d = hx - hy        (chunk-wide tensor_tensor, 2x mode)
                #   t = z*d            (per-tile tensor_scalar, split DVE/ACT)
                #   h = t + hy         (chunk-wide tensor_tensor, 2x mode)
                h16 = out_p.tile([128, NTILE, 3 * D], MM_DT, tag="h16")
                n_ts = 0
                for o, (zi, xi, yi) in enumerate(PAIRS):
                    hx, hy = hs[xi], hs[yi]
                    dg = t_p.tile([128, NTILE * D], MM_DT, tag="d")
                    nc.vector.tensor_sub(dg, hx, hy)
                    tg = t_p.tile([128, NTILE * D], MM_DT, tag="t")
                    for j in range(NTILE):
                        zcol = z_sb[:, j * 3 + o: j * 3 + o + 1]
                        route = TS_ROUTE[n_ts % len(TS_ROUTE)]
                        if route == "a":
                            nc.scalar.activation(
                                out=tg[:, j * D:(j + 1) * D],
                                in_=dg[:, j * D:(j + 1) * D],
                                func=AF.Copy, scale=zcol)
                        elif route == "p":
                            nc.gpsimd.tensor_scalar_mul(
                                tg[:, j * D:(j + 1) * D],
                                dg[:, j * D:(j + 1) * D],
                                zcol)
                        else:
                            nc.vector.tensor_scalar_mul(
                                tg[:, j * D:(j + 1) * D],
                                dg[:, j * D:(j + 1) * D],
                                zcol)
                        n_ts += 1
                    nc.vector.tensor_add(
                        h16[:, :, o * D:(o + 1) * D],
                        tg.rearrange("p (j d) -> p j d", d=D),
                        hy.rearrange("p (j d) -> p j d", d=D))

                nc.sync.dma_start(
                    out=out[csl, :].rearrange("(j p) c -> p j c", p=128),
                    in_=h16)

    nc.compile()
    return nc


_CACHE = {}


def _get_nc(with_bias: bool):
    key = ("nc", with_bias)
    if key not in _CACHE:
        _CACHE[key] = _build_nc(with_bias)
    return _CACHE[key]


def _prep_weights(Wa, Wv, Wl, Wav, Wal, Wvl):
    # w_main[m, kh] = W.T[kh*128:(kh+1)*128, :]  ([128, D] slice of [K, M])
    wm = np.stack([
        np.ascontiguousarray(W.T.reshape(2, 128, D))
        for W in (Wa, Wv, Wl)
    ]).astype(NP_MM_DT)                               # [3, 2, 128, D]
    # gate vectors, split into per-stream blocks of 3 columns
    wav, wal, wvl = Wav[0], Wal[0], Wvl[0]            # (768,)
    Z = np.zeros(D, np.float32)
    blocks = [
        (wav[0:D],      wal[0:D],      Z),            # stream na
        (wav[D:2 * D],  Z,             wvl[0:D]),     # stream nv
        (Z,             wal[D:2 * D],  wvl[D:2 * D]),  # stream nl
        (wav[2 * D:],   Z,             Z),            # stream na*nv
        (Z,             wal[2 * D:],   Z),            # stream na*nl
        (Z,             Z,             wvl[2 * D:]),  # stream nv*nl
    ]
    wg = np.stack([
        np.stack([np.asarray(c0), np.asarray(c1), np.asarray(c2)], axis=1)
        .reshape(2, 128, 3)
        for (c0, c1, c2) in blocks
    ]).astype(NP_MM_DT)                               # [6, 2, 128, 3]
    return wm, wg


def _prep_acts(x, c):
    """x: (L, B, D) fp32 -> core-c feature-major [2, 128, N_CORE] fp16."""
    xc = x[:, c * B_CORE:(c + 1) * B_CORE, :]         # (L, 16, D)
    xt = np.ascontiguousarray(xc.astype(NP_MM_DT).transpose(2, 1, 0))
    return xt.reshape(2, 128, N_CORE)                 # k-major, n = b*L + t


def kernel(**inputs) -> np.ndarray:
    a = np.asarray(inputs["a"], np.float32)
    v = np.asarray(inputs["v"], np.float32)
    l = np.asarray(inputs["l"], np.float32)
    names = ("Wa", "Wv", "Wl", "Wav", "Wal", "Wvl")
    Wa, Wv, Wl, Wav, Wal, Wvl = (np.asarray(inputs[n], np.float32)
                                 for n in names)
    biases = {n: np.asarray(inputs[n], np.float32)
              for n in ("ba", "bv", "bl", "bav", "bal", "bvl")}
    with_bias = any(np.any(b) for b in biases.values())

    nc = _get_nc(with_bias)
    wm, wg = _prep_weights(Wa, Wv, Wl, Wav, Wal, Wvl)

    in_maps = []
    for c in range(N_CORES):
        m = {
            "a_t": _prep_acts(a, c),
            "v_t": _prep_acts(v, c),
            "l_t": _prep_acts(l, c),
            "w_main": wm,
            "w_gate": wg,
        }
        if with_bias:
            m["b_main"] = np.stack(
                [biases["ba"], biases["bv"], biases["bl"]])[None].astype(NP_MM_DT)
            m["b_gate"] = np.array(
                [[biases["bav"][0], biases["bal"][0], biases["bvl"][0]]],
                NP_MM_DT)
        in_maps.append(m)

    trace = bool(int(os.environ.get("KERNEL_TRACE", "0")))
    kw = {}
    if trace and os.environ.get("KERNEL_TRACE_DIR"):
        kw["tmpdir"] = os.environ["KERNEL_TRACE_DIR"]
    res = run_bass_kernel_spmd(nc, in_maps, core_ids=list(range(N_CORES)),
                               trace=trace, **kw)
    _CACHE["last_results"] = res
    return np.concatenate([res.results[c]["out"] for c in range(N_CORES)],
                          axis=0).astype(np.float32)
